# revision 2
# baseline (speedup 1.0000x reference)
"""Depth-aware 3x3 convolution on 8 Trainium2 NeuronCores (Bass, raw engine blocks).

out[b,o,h,w] = sum_{c,kh,kw} weight[o,c,kh,kw] * x[b,c,h+kh-1,w+kw-1]
                             * exp(-8.3*|depth[b,h,w] - depth[b,h+kh-1,w+kw-1]|)

Sharding: core = 2*b + (h >= 128); each core computes a [32, 128, 256] output
slab from a 130-row padded input frame (1-row halo from the host slice).

Datapath is bf16 (x, weight, sim, modulated product, output) with f32 depth
and f32 PSUM accumulation; the DVE modulation multiply runs in 2x perf mode.

Per-core pipeline:
  A. sim: depth rows pixel-major [128, 258]x3 -> sub (DVE) -> |.| (DVE STT)
     -> exp (ACT, bf16) -> DRAM simd[9, 32768]
  B. main loop over 16 tiles of 2048 px (8 rows):
     - DMA: x3 chunk [96, 10*256] bf16 (3 column-shift blocks, pitch-256,
       pre-shifted on host so every slice is 4B-aligned)
     - DMA: broadcast simd rows across 32 partitions -> simrep3 [96, 2048] bf16
     - DVE: xm3 = x3[:, t*256 : t*256+2048] * simrep3  (bf16, 2x mode) x3 t-passes
     - PE : psum[32, 2048] += w3[:, t].T @ xm3  (K=96, N=512 x4, bf16)
     - ACT: psum -> out_sb bf16; DMA out.

The builder takes `iters` to unroll the whole program N times inside one NEFF
(idempotent re-execution) so device time can be measured as a wall-clock slope
without NTFF profiling. The graded path uses iters=1.
"""
import sys

import numpy as np

sys.path.insert(0, "/opt/trn_rl_repo")

import concourse.bass as bass
import concourse.mybir as mybir
from concourse.bass_utils import run_bass_kernel_spmd

F32 = mybir.dt.float32
BF16 = mybir.dt.bfloat16
EXP = mybir.ActivationFunctionType.Exp

B, C, H, W = 4, 32, 256, 256
O = 32
ALPHA = 8.3
R = 128  # output rows per core
WP = W + 2  # padded width (depth frame only)
FR = R + 2  # frame rows per core
NPIX = R * W  # 32768
TROWS = 8  # rows per tile
TILE = TROWS * W  # 2048
NT = R // TROWS  # 16
CH_ROWS = TROWS + 2  # x3 chunk rows
CH = CH_ROWS * W  # x3 chunk elems per partition
MMN = 512  # matmul free-dim chunk
QN = TILE // MMN  # 4


def build_nc(iters=1):
    nc = bass.Bass("TRN2", target_bir_lowering=False, debug=False, num_devices=8)
    # x3: 3 column-shift blocks stacked on partitions, pitch-256 rows
    x3_in = nc.declare_dram_parameter("x3", [96, FR * W], BF16, isOutput=False)
    dp_in = nc.declare_dram_parameter("dp", [FR, WP], F32, isOutput=False)
    w3_in = nc.declare_dram_parameter("w3", [96, 96], BF16, isOutput=False)
    out_d = nc.declare_dram_parameter("out", [O, NPIX], BF16, isOutput=True)
    simd = nc.dram_tensor("simd", [9, NPIX], BF16)
    simd_r = simd.ap().rearrange("k (r w) -> k r w", r=R)

    from contextlib import ExitStack

    ctx = ExitStack()
    with ctx:
        d_sb = ctx.enter_context(nc.sbuf_tensor([128, 3 * WP], F32))
        adiff9 = ctx.enter_context(nc.sbuf_tensor([128, 9 * W], F32))
        sim9 = ctx.enter_context(nc.sbuf_tensor([128, 9 * W], BF16))
        w3_sb = ctx.enter_context(nc.sbuf_tensor([96, 96], BF16))
        x3c = ctx.enter_context(nc.sbuf_tensor([96, 2 * CH], BF16))
        simrep3 = ctx.enter_context(nc.sbuf_tensor([96, 2 * TILE], BF16))
        xm3 = ctx.enter_context(nc.sbuf_tensor([96, 2 * TILE], BF16))
        out_sb = ctx.enter_context(nc.sbuf_tensor([32, 2 * TILE], BF16))
        psum = ctx.enter_context(nc.psum_tensor([32, 2 * TILE], F32))
        ld_sem = ctx.enter_context(nc.semaphore("ld_sem"))
        x_e = ctx.enter_context(nc.semaphore("x_e"))
        x_o = ctx.enter_context(nc.semaphore("x_o"))
        sim_dve = ctx.enter_context(nc.semaphore("sim_dve"))
        act_exp = ctx.enter_context(nc.semaphore("act_exp"))
        sim_st = ctx.enter_context(nc.semaphore("sim_st"))
        bc_e = ctx.enter_context(nc.semaphore("bc_e"))
        bc_o = ctx.enter_context(nc.semaphore("bc_o"))
        mod_sem = ctx.enter_context(nc.semaphore("mod_sem"))
        pe_sem = ctx.enter_context(nc.semaphore("pe_sem"))
        act_cp = ctx.enter_context(nc.semaphore("act_cp"))
        st_e = ctx.enter_context(nc.semaphore("st_e"))
        st_o = ctx.enter_context(nc.semaphore("st_o"))
        block = ctx.enter_context(nc.Block())

        NG = NT * iters  # total tile steps

        @block.sync
        def _(sync: bass.BassEngine):
            for n in range(iters):
                # startup loads: d (3 row-shifted views), w3.  On re-execution
                # wait for the previous iteration's sim phase to release d_sb.
                if n >= 1:
                    sync.wait_ge(sim_dve, 9 * n)
                for t in range(3):
                    sync.dma_start(
                        d_sb[:, t * WP : (t + 1) * WP], dp_in[t : t + 128, :]
                    ).then_inc(ld_sem, 16)
                sync.dma_start(w3_sb[:], w3_in[:]).then_inc(ld_sem, 16)
                # sim -> DRAM (idempotent; gate on prev iter's mods consuming
                # the broadcast buffers before overwriting simd)
                if n >= 1:
                    sync.wait_ge(mod_sem, 48 * n)
                for k in range(9):
                    sync.wait_ge(act_exp, 9 * n + k + 1)
                    sync.dma_start(
                        simd_r[k], sim9[:, k * W : (k + 1) * W]
                    ).then_inc(sim_st, 16)
                # main loop (tile step G is continuous across iterations)
                for i in range(NT):
                    G = NT * n + i
                    bi = G % 2
                    # x3 chunk for tile i
                    if G >= 2:
                        sync.wait_ge(mod_sem, 3 * (G - 2) + 3)
                    sync.dma_start(
                        x3c[:, bi * CH : (bi + 1) * CH],
                        x3_in[:, i * TROWS * W : i * TROWS * W + CH],
                    ).then_inc(x_e if bi == 0 else x_o, 16)
                    # broadcast sim rows for the 3 passes
                    if i == 0:
                        sync.wait_ge(sim_st, 144 * (n + 1))
                    for t in range(3):
                        s = 3 * G + t
                        sb = s % 2
                        if s >= 2:
                            sync.wait_ge(mod_sem, s - 1)
                        for j in range(3):
                            sync.dma_start(
                                simrep3[
                                    32 * j : 32 * (j + 1),
                                    sb * TILE : (sb + 1) * TILE,
                                ],
                                simd[
                                    3 * t + j : 3 * t + j + 1,
                                    i * TILE : (i + 1) * TILE,
                                ].to_broadcast((32, TILE)),
                            ).then_inc(bc_e if sb == 0 else bc_o, 16)
                    # store tile G-1
                    if G >= 1:
                        sync.wait_ge(act_cp, G)
                        sync.dma_start(
                            out_d[:, ((G - 1) % NT) * TILE : (((G - 1) % NT) + 1) * TILE],
                            out_sb[:, ((G - 1) % 2) * TILE : ((G - 1) % 2 + 1) * TILE],
                        ).then_inc(st_e if (G - 1) % 2 == 0 else st_o, 16)
            sync.wait_ge(act_cp, NG)
            sync.dma_start(
                out_d[:, (NT - 1) * TILE :],
                out_sb[:, ((NG - 1) % 2) * TILE : ((NG - 1) % 2 + 1) * TILE],
            ).then_inc(st_e if (NG - 1) % 2 == 0 else st_o, 16)

        @block.vector
        def _(vector):
            for n in range(iters):
                # sim phase: diff + abs per tap
                vector.wait_ge(ld_sem, 64 * (n + 1))
                if n >= 1:
                    vector.wait_ge(act_exp, 9 * n)
                for t in range(3):
                    for j in range(3):
                        k = 3 * t + j
                        vector.tensor_sub(
                            adiff9[:, k * W : (k + 1) * W],
                            d_sb[:, WP + 1 : WP + 1 + W],
                            d_sb[:, t * WP + j : t * WP + j + W],
                        )
                        vector.drain()
                        vector.scalar_tensor_tensor(
                            adiff9[:, k * W : (k + 1) * W],
                            adiff9[:, k * W : (k + 1) * W],
                            -1.0,
                            adiff9[:, k * W : (k + 1) * W],
                            op0=mybir.AluOpType.mult,
                            op1=mybir.AluOpType.max,
                        ).then_inc(sim_dve, 1)
                # modulation loop
                for i in range(NT):
                    G = NT * n + i
                    bi = G % 2
                    vector.wait_ge(x_e if bi == 0 else x_o, 16 * (G // 2 + 1))
                    for t in range(3):
                        s = 3 * G + t
                        sb = s % 2
                        vector.wait_ge(bc_e if sb == 0 else bc_o, 48 * (s // 2 + 1))
                        if s >= 2:
                            vector.wait_ge(pe_sem, s - 1)
                        vector.tensor_mul(
                            xm3[:, sb * TILE : (sb + 1) * TILE],
                            x3c[:, bi * CH + t * W : bi * CH + t * W + TILE],
                            simrep3[:, sb * TILE : (sb + 1) * TILE],
                        ).then_inc(mod_sem, 1)

        @block.tensor
        def _(tensor):
            tensor.wait_ge(ld_sem, 64)
            for G in range(NG):
                bi = G % 2
                if G >= 2:
                    tensor.wait_ge(act_cp, G - 1)
                for t in range(3):
                    s = 3 * G + t
                    sb = s % 2
                    tensor.wait_ge(mod_sem, s + 1)
                    for q in range(QN):
                        mm = tensor.matmul(
                            psum[:, bi * TILE + q * MMN : bi * TILE + (q + 1) * MMN],
                            w3_sb[:, 32 * t : 32 * (t + 1)],
                            xm3[:, sb * TILE + q * MMN : sb * TILE + (q + 1) * MMN],
                            start=(t == 0),
                            stop=(t == 2),
                        )
                        if q == QN - 1:
                            mm.then_inc(pe_sem, 1)

        @block.scalar
        def _(scalar):
            for n in range(iters):
                # exp per tap (bf16 out); gate on prev iter's simd stores
                if n >= 1:
                    scalar.wait_ge(sim_st, 144 * n)
                for k in range(9):
                    scalar.wait_ge(sim_dve, 9 * n + k + 1)
                    scalar.activation(
                        sim9[:, k * W : (k + 1) * W],
                        adiff9[:, k * W : (k + 1) * W],
                        EXP,
                        scale=-ALPHA,
                    ).then_inc(act_exp, 1)
                # psum -> sbuf copies
                for i in range(NT):
                    G = NT * n + i
                    bi = G % 2
                    scalar.wait_ge(pe_sem, 3 * G + 3)
                    if G >= 2:
                        scalar.wait_ge(st_e if G % 2 == 0 else st_o, 16 * (G // 2))
                    scalar.copy(
                        out_sb[:, bi * TILE : (bi + 1) * TILE],
                        psum[:, bi * TILE : (bi + 1) * TILE],
                    ).then_inc(act_cp, 1)

    return nc


_NC_CACHE = {}


def _get_nc(iters=1):
    if iters not in _NC_CACHE:
        _NC_CACHE[iters] = build_nc(iters)
    return _NC_CACHE[iters]


def _prep_core(x, depth, core):
    import ml_dtypes

    b, half = core // 2, core % 2
    r0 = half * R
    # padded frame [C, FR, WP]: image rows r0-1 .. r0+R, zero-padded
    xpad = np.zeros((C, FR, WP), dtype=np.float32)
    dpad = np.zeros((FR, WP), dtype=np.float32)
    lo, hi = r0 - 1, r0 + R + 1
    slo, shi = max(lo, 0), min(hi, H)
    xpad[:, slo - lo : shi - lo, 1 : 1 + W] = x[b, :, slo:shi, :]
    dpad[slo - lo : shi - lo, 1 : 1 + W] = depth[b, 0, slo:shi, :]
    # x3: 3 column-shift blocks stacked on partitions, pitch-256 (pre-shifted)
    x3 = np.empty((3, C, FR, W), dtype=np.float32)
    x3[0] = xpad[:, :, 0:W]  # j=0: w-1
    x3[1] = xpad[:, :, 1 : 1 + W]  # j=1: w
    x3[2] = xpad[:, :, 2 : 2 + W]  # j=2: w+1
    return {
        "x3": x3.reshape(3 * C, FR * W).astype(ml_dtypes.bfloat16),
        "dp": dpad,
        "w3": None,  # filled by caller (shared)
    }


def _prep_inputs(x, depth, weight):
    import ml_dtypes

    x = np.ascontiguousarray(x, dtype=np.float32)
    depth = np.ascontiguousarray(depth, dtype=np.float32)
    weight = np.ascontiguousarray(weight, dtype=np.float32)
    # w3[32j + c, 32t + o] = weight[o, c, t, j]
    w3 = (
        np.transpose(weight, (3, 1, 2, 0))
        .reshape(96, 96)
        .astype(ml_dtypes.bfloat16)
        .copy()
    )
    in_maps = []
    for core in range(8):
        m = _prep_core(x, depth, core)
        m["w3"] = w3
        in_maps.append(m)
    return in_maps


def kernel(x, depth, weight):
    in_maps = _prep_inputs(x, depth, weight)
    nc = _get_nc(1)
    res = run_bass_kernel_spmd(nc, in_maps, list(range(8)))

    out = np.empty((B, O, H, W), dtype=np.float32)
    for core in range(8):
        b, half = core // 2, core % 2
        out[b, :, half * R : (half + 1) * R, :] = (
            res.results[core]["out"].astype(np.float32).reshape(O, R, W)
        )
    return out


# revision 5
# speedup vs baseline: 11.3414x; 11.3414x over previous
"""Depth-aware 3x3 convolution on 8 Trainium2 NeuronCores (Bass, raw engine blocks).

out[b,o,h,w] = sum_{c,kh,kw} weight[o,c,kh,kw] * x[b,c,h+kh-1,w+kw-1]
                             * exp(-8.3*|depth[b,h,w] - depth[b,h+kh-1,w+kw-1]|)

Sharding: core = 2*b + (h >= 128); each core computes a [32, 128, 256] output
slab from a 130-row padded input frame (1-row halo from the host slice).

Datapath is bf16 (x, weight, sim, modulated product, output) with f32 depth
and f32 PSUM accumulation; the DVE modulation multiply runs in 2x perf mode
(all operands contiguous, 4B-aligned, pitch-256 pre-shifted on the host).

Per-core pipeline:
  A. sim: depth rows pixel-major [128, 258]x3 -> sub (DVE) -> |.| (DVE STT)
     -> exp (ACT, bf16) -> DRAM simd[9, 32768]
  B. main loop over 16 tiles of 2048 px (8 rows):
     - DMA: x3 chunk [96, 10*256] bf16 (3 column-shift blocks on partitions)
     - DMA: broadcast simd rows across 32 partitions -> simrep3 [96, 2048] bf16
     - DVE: xm3 = x3[:, t*256 : t*256+2048] * simrep3  (bf16 2x)  t=0,1,2
     - PE : psum[32, 2048] += w3[:, t].T @ xm3  (K=96, N=512 x4, bf16)
     - ACT: psum -> out_sb bf16; DMA out.

The program body sits in a per-engine hardware loop (`trips`) with a
leader-follower barrier and exact semaphore reset between trips, so the same
NEFF re-executes the kernel N times; device time is then measured as the
wall-clock slope between two trip counts.  The graded path uses trips=1.
"""
import sys

import numpy as np

sys.path.insert(0, "/opt/trn_rl_repo")

import concourse.bass as bass
import concourse.mybir as mybir
from concourse.bass_utils import run_bass_kernel_spmd

F32 = mybir.dt.float32
BF16 = mybir.dt.bfloat16
EXP = mybir.ActivationFunctionType.Exp

B, C, H, W = 4, 32, 256, 256
O = 32
ALPHA = 8.3
R = 128  # output rows per core
WP = W + 2  # padded width (depth frame only)
FR = R + 2  # frame rows per core
NPIX = R * W  # 32768
TROWS = 8  # rows per tile
TILE = TROWS * W  # 2048
NT = R // TROWS  # 16
CH_ROWS = TROWS + 2  # x3 chunk rows
CH = CH_ROWS * W  # x3 chunk elems per partition
MMN = 512  # matmul free-dim chunk
QN = TILE // MMN  # 4


def build_nc(trips=1):
    nc = bass.Bass("TRN2", target_bir_lowering=False, debug=False, num_devices=8)
    # x3: 3 column-shift blocks stacked on partitions, pitch-256 rows
    x3_in = nc.declare_dram_parameter("x3", [96, FR * W], BF16, isOutput=False)
    dp_in = nc.declare_dram_parameter("dp", [FR, WP], F32, isOutput=False)
    w3_in = nc.declare_dram_parameter("w3", [96, 96], BF16, isOutput=False)
    out_d = nc.declare_dram_parameter("out", [O, NPIX], BF16, isOutput=True)
    simd = nc.dram_tensor("simd", [9, NPIX], BF16)
    simd_r = simd.ap().rearrange("k (r w) -> k r w", r=R)

    from contextlib import ExitStack

    ctx = ExitStack()
    with ctx:
        d_sb = ctx.enter_context(nc.sbuf_tensor([128, 3 * WP], F32))
        adiff9 = ctx.enter_context(nc.sbuf_tensor([128, 9 * W], F32))
        sim9 = ctx.enter_context(nc.sbuf_tensor([128, 9 * W], BF16))
        w3_sb = ctx.enter_context(nc.sbuf_tensor([96, 96], BF16))
        x3c = ctx.enter_context(nc.sbuf_tensor([96, 2 * CH], BF16))
        simrep3 = ctx.enter_context(nc.sbuf_tensor([96, 2 * TILE], BF16))
        xm3 = ctx.enter_context(nc.sbuf_tensor([96, 2 * TILE], BF16))
        out_sb = ctx.enter_context(nc.sbuf_tensor([32, 2 * TILE], BF16))
        psum = ctx.enter_context(nc.psum_tensor([32, 2 * TILE], F32))
        ld_sem = ctx.enter_context(nc.semaphore("ld_sem"))
        x_e = ctx.enter_context(nc.semaphore("x_e"))
        x_o = ctx.enter_context(nc.semaphore("x_o"))
        sim_dve = ctx.enter_context(nc.semaphore("sim_dve"))
        act_exp = ctx.enter_context(nc.semaphore("act_exp"))
        sim_st = ctx.enter_context(nc.semaphore("sim_st"))
        bc_e = ctx.enter_context(nc.semaphore("bc_e"))
        bc_o = ctx.enter_context(nc.semaphore("bc_o"))
        mod_sem = ctx.enter_context(nc.semaphore("mod_sem"))
        pe_sem = ctx.enter_context(nc.semaphore("pe_sem"))
        act_cp = ctx.enter_context(nc.semaphore("act_cp"))
        st_e = ctx.enter_context(nc.semaphore("st_e"))
        st_o = ctx.enter_context(nc.semaphore("st_o"))
        bar_g = ctx.enter_context(nc.semaphore("bar_g"))
        bar_r = ctx.enter_context(nc.semaphore("bar_r"))
        bar_a = ctx.enter_context(nc.semaphore("bar_a"))
        bar_r2 = ctx.enter_context(nc.semaphore("bar_r2"))
        block = ctx.enter_context(nc.Block())

        PIPE_SEMS = [
            ld_sem, sim_dve, act_exp, sim_st, x_e, x_o, bc_e, bc_o,
            mod_sem, pe_sem, act_cp, st_e, st_o,
        ]

        def follower_barrier(eng):
            # two-phase: park on bar_r while SP resets pipe sems, then ack and
            # park on bar_r2 while SP resets bar_r.  All wait values are
            # trip-invariant; every sem returns to 0 each trip.
            eng.drain()
            eng.sem_inc(bar_g, 1)
            eng.wait_ge(bar_r, 1)
            eng.sem_inc(bar_a, 1)
            eng.wait_ge(bar_r2, 1)

        @block.sync
        def _(sync: bass.BassEngine):
            with sync.Fori(0, trips):
                # startup loads: d (3 row-shifted views), w3
                for t in range(3):
                    sync.dma_start(
                        d_sb[:, t * WP : (t + 1) * WP], dp_in[t : t + 128, :]
                    ).then_inc(ld_sem, 16)
                sync.dma_start(w3_sb[:], w3_in[:]).then_inc(ld_sem, 16)
                # sim -> DRAM
                for k in range(9):
                    sync.wait_ge(act_exp, k + 1)
                    sync.dma_start(
                        simd_r[k], sim9[:, k * W : (k + 1) * W]
                    ).then_inc(sim_st, 16)
                # main loop
                for i in range(NT):
                    bi = i % 2
                    # x3 chunk for tile i
                    if i >= 2:
                        sync.wait_ge(mod_sem, 3 * (i - 2) + 3)
                    sync.dma_start(
                        x3c[:, bi * CH : (bi + 1) * CH],
                        x3_in[:, i * TROWS * W : i * TROWS * W + CH],
                    ).then_inc(x_e if bi == 0 else x_o, 16)
                    # broadcast sim rows for the 3 passes
                    if i == 0:
                        sync.wait_ge(sim_st, 144)
                    for t in range(3):
                        s = 3 * i + t
                        sb = s % 2
                        if s >= 2:
                            sync.wait_ge(mod_sem, s - 1)
                        for j in range(3):
                            sync.dma_start(
                                simrep3[
                                    32 * j : 32 * (j + 1),
                                    sb * TILE : (sb + 1) * TILE,
                                ],
                                simd[
                                    3 * t + j : 3 * t + j + 1,
                                    i * TILE : (i + 1) * TILE,
                                ].to_broadcast((32, TILE)),
                            ).then_inc(bc_e if sb == 0 else bc_o, 16)
                    # store tile i-1
                    if i >= 1:
                        sync.wait_ge(act_cp, i)
                        sync.dma_start(
                            out_d[:, (i - 1) * TILE : i * TILE],
                            out_sb[:, ((i - 1) % 2) * TILE : ((i - 1) % 2 + 1) * TILE],
                        ).then_inc(st_e if (i - 1) % 2 == 0 else st_o, 16)
                sync.wait_ge(act_cp, NT)
                sync.dma_start(
                    out_d[:, (NT - 1) * TILE :],
                    out_sb[:, ((NT - 1) % 2) * TILE : ((NT - 1) % 2 + 1) * TILE],
                ).then_inc(st_e if (NT - 1) % 2 == 0 else st_o, 16)
                # ---- trip barrier: leader ----
                # all DMA completions at their exact per-trip finals
                sync.wait_ge(ld_sem, 64)
                sync.wait_ge(sim_st, 144)
                sync.wait_ge(x_e, 128)
                sync.wait_ge(x_o, 128)
                sync.wait_ge(bc_e, 1152)
                sync.wait_ge(bc_o, 1152)
                sync.wait_ge(st_e, 128)
                sync.wait_ge(st_o, 128)
                # phase 1: engines idle (parked on bar_r); reset pipe sems
                sync.wait_ge(bar_g, 3)
                for sem in PIPE_SEMS:
                    sync.sem_clear(sem)
                sync.sem_clear(bar_g)
                sync.sem_clear(bar_r2)
                sync.sem_inc(bar_r, 1)
                # phase 2: engines parked on bar_r2; reset bar_r and release
                sync.wait_ge(bar_a, 3)
                sync.sem_clear(bar_r)
                sync.sem_clear(bar_a)
                sync.sem_inc(bar_r2, 1)

        @block.vector
        def _(vector):
            with vector.Fori(0, trips):
                # sim phase: diff + abs per tap
                vector.wait_ge(ld_sem, 64)
                for t in range(3):
                    for j in range(3):
                        k = 3 * t + j
                        vector.tensor_sub(
                            adiff9[:, k * W : (k + 1) * W],
                            d_sb[:, WP + 1 : WP + 1 + W],
                            d_sb[:, t * WP + j : t * WP + j + W],
                        )
                        vector.drain()
                        vector.scalar_tensor_tensor(
                            adiff9[:, k * W : (k + 1) * W],
                            adiff9[:, k * W : (k + 1) * W],
                            -1.0,
                            adiff9[:, k * W : (k + 1) * W],
                            op0=mybir.AluOpType.mult,
                            op1=mybir.AluOpType.max,
                        ).then_inc(sim_dve, 1)
                # modulation loop
                for i in range(NT):
                    bi = i % 2
                    vector.wait_ge(x_e if bi == 0 else x_o, 16 * (i // 2 + 1))
                    for t in range(3):
                        s = 3 * i + t
                        sb = s % 2
                        vector.wait_ge(bc_e if sb == 0 else bc_o, 48 * (s // 2 + 1))
                        if s >= 2:
                            vector.wait_ge(pe_sem, s - 1)
                        vector.tensor_mul(
                            xm3[:, sb * TILE : (sb + 1) * TILE],
                            x3c[:, bi * CH + t * W : bi * CH + t * W + TILE],
                            simrep3[:, sb * TILE : (sb + 1) * TILE],
                        ).then_inc(mod_sem, 1)
                follower_barrier(vector)

        @block.tensor
        def _(tensor):
            with tensor.Fori(0, trips):
                tensor.wait_ge(ld_sem, 64)
                for i in range(NT):
                    bi = i % 2
                    if i >= 2:
                        tensor.wait_ge(act_cp, i - 1)
                    for t in range(3):
                        s = 3 * i + t
                        sb = s % 2
                        tensor.wait_ge(mod_sem, s + 1)
                        for q in range(QN):
                            mm = tensor.matmul(
                                psum[
                                    :, bi * TILE + q * MMN : bi * TILE + (q + 1) * MMN
                                ],
                                w3_sb[:, 32 * t : 32 * (t + 1)],
                                xm3[:, sb * TILE + q * MMN : sb * TILE + (q + 1) * MMN],
                                start=(t == 0),
                                stop=(t == 2),
                            )
                            if q == QN - 1:
                                mm.then_inc(pe_sem, 1)
                follower_barrier(tensor)

        @block.scalar
        def _(scalar):
            with scalar.Fori(0, trips):
                # exp per tap (bf16 out)
                for k in range(9):
                    scalar.wait_ge(sim_dve, k + 1)
                    scalar.activation(
                        sim9[:, k * W : (k + 1) * W],
                        adiff9[:, k * W : (k + 1) * W],
                        EXP,
                        scale=-ALPHA,
                    ).then_inc(act_exp, 1)
                # psum -> sbuf copies
                for i in range(NT):
                    bi = i % 2
                    scalar.wait_ge(pe_sem, 3 * i + 3)
                    if i >= 2:
                        scalar.wait_ge(st_e if i % 2 == 0 else st_o, 16 * (i // 2))
                    scalar.copy(
                        out_sb[:, bi * TILE : (bi + 1) * TILE],
                        psum[:, bi * TILE : (bi + 1) * TILE],
                    ).then_inc(act_cp, 1)
                follower_barrier(scalar)

    return nc


_NC_CACHE = {}


def _get_nc(trips=1):
    if trips not in _NC_CACHE:
        _NC_CACHE[trips] = build_nc(trips)
    return _NC_CACHE[trips]


def _prep_core(x, depth, core):
    import ml_dtypes

    b, half = core // 2, core % 2
    r0 = half * R
    # padded frame [C, FR, WP]: image rows r0-1 .. r0+R, zero-padded
    xpad = np.zeros((C, FR, WP), dtype=np.float32)
    dpad = np.zeros((FR, WP), dtype=np.float32)
    lo, hi = r0 - 1, r0 + R + 1
    slo, shi = max(lo, 0), min(hi, H)
    xpad[:, slo - lo : shi - lo, 1 : 1 + W] = x[b, :, slo:shi, :]
    dpad[slo - lo : shi - lo, 1 : 1 + W] = depth[b, 0, slo:shi, :]
    # x3: 3 column-shift blocks stacked on partitions, pitch-256 (pre-shifted)
    x3 = np.empty((3, C, FR, W), dtype=np.float32)
    x3[0] = xpad[:, :, 0:W]  # j=0: w-1
    x3[1] = xpad[:, :, 1 : 1 + W]  # j=1: w
    x3[2] = xpad[:, :, 2 : 2 + W]  # j=2: w+1
    return {
        "x3": x3.reshape(3 * C, FR * W).astype(ml_dtypes.bfloat16),
        "dp": dpad,
        "w3": None,  # filled by caller (shared)
    }


def _prep_inputs(x, depth, weight):
    import ml_dtypes

    x = np.ascontiguousarray(x, dtype=np.float32)
    depth = np.ascontiguousarray(depth, dtype=np.float32)
    weight = np.ascontiguousarray(weight, dtype=np.float32)
    # w3[32j + c, 32t + o] = weight[o, c, t, j]
    w3 = (
        np.transpose(weight, (3, 1, 2, 0))
        .reshape(96, 96)
        .astype(ml_dtypes.bfloat16)
        .copy()
    )
    in_maps = []
    for core in range(8):
        m = _prep_core(x, depth, core)
        m["w3"] = w3
        in_maps.append(m)
    return in_maps


def kernel(x, depth, weight):
    in_maps = _prep_inputs(x, depth, weight)
    nc = _get_nc(1)
    res = run_bass_kernel_spmd(nc, in_maps, list(range(8)))

    out = np.empty((B, O, H, W), dtype=np.float32)
    for core in range(8):
        b, half = core // 2, core % 2
        out[b, :, half * R : (half + 1) * R, :] = (
            res.results[core]["out"].astype(np.float32).reshape(O, R, W)
        )
    return out
